# revision 1
# baseline (speedup 1.0000x reference)
"""ContactAwareLoss Trainium2 kernel.

Strategy: pure data-parallel over batch (512 rows -> 8 cores x 64 rows).
Each core computes four partial sums over its shard:
  [0] sum_{t,h} probs2 * |dist - 0.1|            (contact distance, unnormalized)
  [1] sum_{j,h} probs2[j+1] * ||r[j+1]-r[j]||     (contact velocity, unnormalized)
  [2] sum_{t,h} first_contact * (5-tap sum of |second diff of dist|)
  [3] sum first_contact                           (count)
The host divides by the global element counts / count and applies the ramp.

On-chip layout: partition p = half*64 + b  (sequence halved so 64 batch rows
fill 128 partitions); free dim = time within the half, processed in W-wide
chunks with a 3-element halo on both sides.  The halo at the half boundary is
filled with real neighbour data via small extra DMAs; the halo at the global
sequence ends is zero-filled and the affected contributions are masked by
zeroing q/vd edge columns (smoothness valid t in [3, seq-3), velocity valid
j in [0, seq-1)).

Engine split:
 - DMA: hand+obj on the sync HWDGE ring, probs on the scalar HWDGE ring
   (both fp32 - SWDGE cast DMAs measured ~75 GB/s, far slower than fp32
   HWDGE, so the bf16 conversion rides the compute ops' output dtype).
 - DVE: r (fp32->bf16), c-sums, diffs/movsum (bf16 2x mode - all time shifts
   in the (t, h*c)-major layouts are 4-byte aligned), fused weighted-sum
   accumulators (scalar_tensor_tensor).
 - ScalarE: Square / Sqrt / Abs (contiguous APs only - strided activation
   outputs measured 5x slow).
 - GpSimd: first-contact mask pipeline (cb/fc+count) to offload the DVE.
"""

import numpy as np

BS, SEQ = 512, 4096
N_CORES = 8
W_FULL = 512  # chunk width (per half-sequence)


def build_nc(bs_local, seq, W):
    import concourse.bass as bass
    import concourse.bacc as bacc
    import concourse.tile as tile
    from concourse import mybir

    f32 = mybir.dt.float32
    bf16 = mybir.dt.bfloat16
    Alu = mybir.AluOpType
    Act = mybir.ActivationFunctionType

    P = 2 * bs_local          # partitions used
    HS = seq // 2             # timesteps per partition row
    assert HS % W == 0
    C = HS // W               # chunks
    E = W + 6                 # chunk width incl. +-3 halo
    H = P // 2

    nc = bacc.Bacc("TRN2", target_bir_lowering=False, debug=False)
    hand = nc.dram_tensor("pred_hand_pos", [bs_local, seq, 2, 3], f32, kind="ExternalInput")
    obj = nc.dram_tensor("pred_obj_pos", [bs_local, seq, 3], f32, kind="ExternalInput")
    probs = nc.dram_tensor("contact_probs", [bs_local, seq, 3], f32, kind="ExternalInput")
    partials = nc.dram_tensor("partials", [P, 4], f32, kind="ExternalOutput")

    def dram_ap(t, offset, dims):
        return bass.AP(tensor=t, offset=offset, ap=[list(d) for d in dims])

    with tile.TileContext(nc) as tc:
        import contextlib
        with contextlib.ExitStack() as ctx:
            inp = ctx.enter_context(tc.tile_pool(name="inp", bufs=2))
            work = ctx.enter_context(tc.tile_pool(name="work", bufs=1))
            singles = ctx.enter_context(tc.tile_pool(name="singles", bufs=1))

            l1s = singles.tile([P, C], f32)
            l2s = singles.tile([P, C], f32)
            sms = singles.tile([P, C], f32)
            cns = singles.tile([P, C], f32)
            outt = singles.tile([P, 4], f32)
            c_neg01 = singles.tile([P, 1], f32)
            nc.vector.memset(c_neg01[:], -0.1)

            for c in range(C):
                t0 = c * W  # first owned timestep (within half)
                t_lo = max(0, t0 - 3)
                t_hi = min(HS, t0 + W + 3)
                col_lo = t_lo - (t0 - 3)
                ncols = t_hi - t_lo

                hand_t = inp.tile([P, E, 6], f32)
                obj_t = inp.tile([P, E, 3], f32)
                probs_t = inp.tile([P, E, 3], f32)

                loads = (
                    (hand_t, hand, 6, nc.sync),
                    (obj_t, obj, 3, nc.sync),
                    (probs_t, probs, 3, nc.scalar),
                )
                for tile_buf, ten, k, eng in loads:
                    eng.dma_start(
                        out=tile_buf[:, col_lo:col_lo + ncols, :],
                        in_=dram_ap(ten, t_lo * k,
                                    [[HS * k, 2], [seq * k, bs_local], [1, ncols * k]]),
                    )
                    if c == 0:
                        eng.dma_start(
                            out=tile_buf[H:P, 0:3, :],
                            in_=dram_ap(ten, (HS - 3) * k,
                                        [[seq * k, bs_local], [1, 3 * k]]),
                        )
                        nc.vector.memset(tile_buf[0:H, 0:3, :], 0.0)
                    if c == C - 1:
                        eng.dma_start(
                            out=tile_buf[0:H, W + 3:E, :],
                            in_=dram_ap(ten, HS * k,
                                        [[seq * k, bs_local], [1, 3 * k]]),
                        )
                        nc.vector.memset(tile_buf[H:P, W + 3:E, :], 0.0)

                # ---- r = hand - obj (one strided sub per hand, fp32 -> bf16) ----
                r_t = work.tile([P, E, 6], bf16)
                for h in range(2):
                    nc.vector.tensor_sub(r_t[:, :, 3 * h:3 * h + 3],
                                         hand_t[:, :, 3 * h:3 * h + 3], obj_t[:])

                # ---- d2 = sum_c r^2 (Square on ACT, two strided adds) ----
                sq_t = work.tile([P, E, 6], bf16)
                nc.scalar.activation(sq_t[:], r_t[:], Act.Square)
                sqa = sq_t[:]

                def csum(dst, src_ap, n):
                    """dst[t,h] = src[t,3h]+src[t,3h+1]+src[t,3h+2] over n positions."""
                    v = [bass.AP(tensor=src_ap.tensor, offset=src_ap.offset + cc,
                                 ap=[src_ap.ap[0], [3, 2 * n]]) for cc in range(3)]
                    tmp = work.tile([P, n, 2], bf16, tag=f"csum_tmp")
                    ta = bass.AP(tensor=tmp.tensor, offset=tmp[:].offset,
                                 ap=[tmp[:].ap[0], [1, 2 * n]])
                    nc.vector.tensor_add(ta, v[0], v[1])
                    nc.vector.tensor_add(dst, ta, v[2])

                d2_t = work.tile([P, E, 2], bf16)
                csum(d2_t[:].opt(), sqa, E)
                d_t = work.tile([P, E, 2], bf16)
                nc.scalar.activation(d_t[:], d2_t[:], Act.Sqrt)

                # ---- contact distance partial ----
                derr_t = work.tile([P, W, 2], bf16)
                nc.scalar.activation(derr_t[:], d_t[:, 3:3 + W, :], Act.Abs, bias=c_neg01[:])
                l1p_t = work.tile([P, W, 2], f32)
                nc.vector.scalar_tensor_tensor(
                    out=l1p_t[:], in0=probs_t[:, 3:3 + W, 0:2], scalar=1.0, in1=derr_t[:],
                    op0=Alu.mult, op1=Alu.mult, accum_out=l1s[:, c:c + 1])

                # ---- velocity ----
                dr_t = work.tile([P, W, 6], bf16)
                nc.vector.tensor_sub(dr_t[:], r_t[:, 4:4 + W, :], r_t[:, 3:3 + W, :])
                dsq_t = work.tile([P, W, 6], bf16)
                nc.scalar.activation(dsq_t[:], dr_t[:], Act.Square)
                v2_t = work.tile([P, W, 2], bf16)
                csum(v2_t[:].opt(), dsq_t[:], W)
                vd_t = work.tile([P, W, 2], bf16)
                nc.scalar.activation(vd_t[:], v2_t[:], Act.Sqrt)
                if c == C - 1:
                    nc.vector.memset(vd_t[H:P, W - 1:W, :], 0.0)  # j=seq-1 invalid
                l2p_t = work.tile([P, W, 2], f32)
                nc.vector.scalar_tensor_tensor(
                    out=l2p_t[:], in0=probs_t[:, 4:4 + W, 0:2], scalar=1.0, in1=vd_t[:],
                    op0=Alu.mult, op1=Alu.mult, accum_out=l2s[:, c:c + 1])

                # ---- smoothness ----
                e_t = work.tile([P, E - 1, 2], bf16)
                nc.vector.tensor_sub(e_t[:], d_t[:, 1:E, :], d_t[:, 0:E - 1, :])
                sdp_t = work.tile([P, W + 4, 2], bf16)
                nc.vector.tensor_sub(sdp_t[:], e_t[:, 0:W + 4, :], e_t[:, 1:W + 5, :])
                sd_t = work.tile([P, W + 4, 2], bf16)
                nc.scalar.activation(sd_t[:], sdp_t[:], Act.Abs)
                s2_t = work.tile([P, W + 3, 2], bf16)
                nc.vector.tensor_add(s2_t[:], sd_t[:, 0:W + 3, :], sd_t[:, 1:W + 4, :])
                s4_t = work.tile([P, W + 1, 2], bf16)
                nc.vector.tensor_add(s4_t[:], s2_t[:, 0:W + 1, :], s2_t[:, 2:W + 3, :])
                sm5_t = work.tile([P, W, 2], bf16)
                nc.vector.tensor_add(sm5_t[:], s4_t[:, 0:W, :], sd_t[:, 4:W + 4, :])

                # ---- first contact mask + count (on GpSimd) ----
                cb_t = work.tile([P, W + 1, 2], bf16)
                nc.gpsimd.tensor_scalar(
                    out=cb_t[:], in0=probs_t[:, 2:3 + W, 0:2],
                    scalar1=0.5, scalar2=None, op0=Alu.is_gt)
                q_t = work.tile([P, W, 2], bf16)
                nc.gpsimd.tensor_sub(q_t[:], cb_t[:, 1:W + 1, :], cb_t[:, 0:W, :])
                if c == 0:
                    nc.vector.memset(q_t[0:H, 0:3, :], 0.0)  # t<3 (incl. forced-false t=0)
                if c == C - 1:
                    nc.vector.memset(q_t[H:P, W - 3:W, :], 0.0)  # t >= seq-3
                fc_t = work.tile([P, W, 2], bf16)
                nc.vector.tensor_scalar(
                    out=fc_t[:], in0=q_t[:], scalar1=0.0, scalar2=0.0,
                    op0=Alu.max, op1=Alu.add, accum_out=cns[:, c:c + 1])

                smp_t = work.tile([P, W, 2], f32)
                nc.vector.scalar_tensor_tensor(
                    out=smp_t[:], in0=sm5_t[:], scalar=1.0, in1=fc_t[:],
                    op0=Alu.mult, op1=Alu.mult, accum_out=sms[:, c:c + 1])

            # ---- final per-partition combine + store ----
            for i, slot in enumerate((l1s, l2s, sms, cns)):
                nc.vector.tensor_reduce(outt[:, i:i + 1], slot[:], axis=mybir.AxisListType.X, op=Alu.add)
            nc.sync.dma_start(out=partials.ap(), in_=outt[:])

    nc.compile()
    return nc


_cache = {}


def _get_nc(bs_local, seq, W):
    key = (bs_local, seq, W)
    if key not in _cache:
        _cache[key] = build_nc(bs_local, seq, W)
    return _cache[key]


def combine_partials(parts, bs, seq, training_step):
    """parts: float array [..., 4] of per-core/per-partition partial sums."""
    s = np.asarray(parts, dtype=np.float64).reshape(-1, 4).sum(axis=0)
    l1 = s[0] / (bs * seq * 2)
    l2 = s[1] / (bs * (seq - 1) * 2) if seq > 1 else 0.0
    cnt = s[3]
    sm = (s[2] / 5.0) / max(cnt, 1.0) if (seq > 5 and cnt > 0) else 0.0
    ramp = min(1.0, float(training_step) / 1000.0)
    return np.array(ramp * (1.0 * l1 + 0.5 * l2 + 0.3 * sm), dtype=np.float32)


def _run(pred_hand_pos, pred_obj_pos, contact_probs, **spmd_kwargs):
    from concourse.bass_utils import run_bass_kernel_spmd

    hand = np.ascontiguousarray(np.asarray(pred_hand_pos, dtype=np.float32))
    obj = np.ascontiguousarray(np.asarray(pred_obj_pos, dtype=np.float32))
    probs = np.ascontiguousarray(np.asarray(contact_probs, dtype=np.float32))
    bs, seq = hand.shape[:2]
    bs_local = bs // N_CORES
    nc = _get_nc(bs_local, seq, W_FULL)

    in_maps = []
    for i in range(N_CORES):
        sl = slice(i * bs_local, (i + 1) * bs_local)
        in_maps.append({
            "pred_hand_pos": hand[sl],
            "pred_obj_pos": obj[sl],
            "contact_probs": probs[sl],
        })
    # The axon terminal occasionally reports the exec unit unrecoverable on
    # the first touch after a previous process's teardown; a retry lands on a
    # recovered device.
    last_err = None
    for _ in range(3):
        try:
            res = run_bass_kernel_spmd(
                nc, in_maps, core_ids=list(range(N_CORES)), **spmd_kwargs
            )
            parts = np.stack([res.results[i]["partials"] for i in range(N_CORES)])
            return parts, res
        except Exception as e:  # noqa: BLE001
            last_err = e
    raise last_err


def kernel(pred_hand_pos, pred_obj_pos, contact_probs, training_step):
    bs, seq = np.asarray(pred_hand_pos).shape[:2]
    parts, _ = _run(pred_hand_pos, pred_obj_pos, contact_probs)
    return combine_partials(parts, bs, seq, training_step)



# revision 5
# speedup vs baseline: 2.0029x; 2.0029x over previous
"""ContactAwareLoss Trainium2 kernel.

Strategy: pure data-parallel over batch (512 rows -> 8 cores x 64 rows).
Each core computes four partial sums over its shard:
  [0] sum_{t,h} probs2 * |dist - 0.1|            (contact distance, unnormalized)
  [1] sum_{j,h} probs2[j+1] * ||r[j+1]-r[j]||     (contact velocity, unnormalized)
  [2] sum_{t,h} first_contact * (5-tap sum of |second diff of dist|)
  [3] sum first_contact                           (count)
The host divides by the global element counts / count and applies the ramp.

On-chip layout: partition p = half*64 + b  (sequence halved so 64 batch rows
fill 128 partitions); free dim = time within the half, processed in W-wide
chunks with a 3-element halo on both sides.  The halo at the half boundary is
filled with real neighbour data via small extra DMAs; the halo at the global
sequence ends is zero-filled and the affected contributions are masked by
zeroing q/vd edge columns (smoothness valid t in [3, seq-3), velocity valid
j in [0, seq-1)).

Engine split (v2):
 - DMA: the three big per-chunk input loads ride SWDGE (nc.gpsimd.dma_start).
   SWDGE sprays descriptors across all 16 SDMA engines (~120+ GB/s measured)
   while the HWDGE rings (sync/scalar) pin all >4KB descriptors to SDMA
   engines 64/65 only (~52 GB/s aggregate).  Halo loads (tiny) stay on the
   HWDGE rings so they don't consume Q7 descriptor-emission throughput.
 - GpSimd: SWDGE descriptor emission only (no compute -- gpsimd tensor ops
   measured ~8 G elem/s, 15x slower than DVE).
 - DVE: diffs/movsum in bf16 2x mode, abs via tensor_scalar abs_max (4x),
   first-contact mask (is_gt), fused weighted-sum accumulators.
 - ScalarE: Square / Sqrt only (contiguous APs).
"""

import numpy as np

BS, SEQ = 512, 4096
N_CORES = 8
W_FULL = 512  # chunk width (per half-sequence)
R_BROADCAST = True  # single hand-obj sub via zero-stride obj AP


def build_nc(bs_local, seq, W):
    import concourse.bass as bass
    import concourse.bacc as bacc
    import concourse.tile as tile
    from concourse import mybir

    f32 = mybir.dt.float32
    bf16 = mybir.dt.bfloat16
    Alu = mybir.AluOpType
    Act = mybir.ActivationFunctionType

    P = 2 * bs_local          # partitions used
    HS = seq // 2             # timesteps per partition row
    assert HS % W == 0
    C = HS // W               # chunks
    E = W + 6                 # chunk width incl. +-3 halo
    H = P // 2

    nc = bacc.Bacc("TRN2", target_bir_lowering=False, debug=False)
    hand = nc.dram_tensor("pred_hand_pos", [bs_local, seq, 2, 3], f32, kind="ExternalInput")
    obj = nc.dram_tensor("pred_obj_pos", [bs_local, seq, 3], f32, kind="ExternalInput")
    probs = nc.dram_tensor("contact_probs", [bs_local, seq, 3], f32, kind="ExternalInput")
    partials = nc.dram_tensor("partials", [P, 4], f32, kind="ExternalOutput")

    def dram_ap(t, offset, dims):
        return bass.AP(tensor=t, offset=offset, ap=[list(d) for d in dims])

    with tile.TileContext(nc) as tc:
        import contextlib
        with contextlib.ExitStack() as ctx:
            inp = ctx.enter_context(tc.tile_pool(name="inp", bufs=2))
            work = ctx.enter_context(tc.tile_pool(name="work", bufs=1))
            singles = ctx.enter_context(tc.tile_pool(name="singles", bufs=1))

            l1s = singles.tile([P, C], f32)
            l2s = singles.tile([P, C], f32)
            sms = singles.tile([P, C], f32)
            cns = singles.tile([P, C], f32)
            outt = singles.tile([P, 4], f32)
            c_neg01 = singles.tile([P, 1], f32)
            nc.vector.memset(c_neg01[:], -0.1)

            for c in range(C):
                t0 = c * W  # first owned timestep (within half)
                t_lo = max(0, t0 - 3)
                t_hi = min(HS, t0 + W + 3)
                col_lo = t_lo - (t0 - 3)
                ncols = t_hi - t_lo

                hand_t = inp.tile([P, E, 6], f32)
                obj_t = inp.tile([P, E, 3], f32)
                probs_t = inp.tile([P, E, 3], f32)

                loads = (
                    (hand_t, hand, 6, nc.sync),
                    (obj_t, obj, 3, nc.sync),
                    (probs_t, probs, 3, nc.scalar),
                )
                for tile_buf, ten, k, halo_eng in loads:
                    # big main load: SWDGE (sprays all 16 SDMA engines)
                    nc.gpsimd.dma_start(
                        out=tile_buf[:, col_lo:col_lo + ncols, :],
                        in_=dram_ap(ten, t_lo * k,
                                    [[HS * k, 2], [seq * k, bs_local], [1, ncols * k]]),
                    )
                    if c == 0:
                        halo_eng.dma_start(
                            out=tile_buf[H:P, 0:3, :],
                            in_=dram_ap(ten, (HS - 3) * k,
                                        [[seq * k, bs_local], [1, 3 * k]]),
                        )
                        nc.vector.memset(tile_buf[0:H, 0:3, :], 0.0)
                    if c == C - 1:
                        halo_eng.dma_start(
                            out=tile_buf[0:H, W + 3:E, :],
                            in_=dram_ap(ten, HS * k,
                                        [[seq * k, bs_local], [1, 3 * k]]),
                        )
                        nc.vector.memset(tile_buf[H:P, W + 3:E, :], 0.0)

                # ---- r = hand - obj (fp32 -> bf16) ----
                r_t = work.tile([P, E, 6], bf16)
                if R_BROADCAST:
                    ha = hand_t[:]
                    ra = r_t[:]
                    oa = obj_t[:]
                    hand_v = bass.AP(tensor=ha.tensor, offset=ha.offset,
                                     ap=[ha.ap[0], [6, E], [3, 2], [1, 3]])
                    r_v = bass.AP(tensor=ra.tensor, offset=ra.offset,
                                  ap=[ra.ap[0], [6, E], [3, 2], [1, 3]])
                    obj_v = bass.AP(tensor=oa.tensor, offset=oa.offset,
                                    ap=[oa.ap[0], [3, E], [0, 2], [1, 3]])
                    nc.vector.tensor_sub(r_v, hand_v, obj_v)
                else:
                    for h in range(2):
                        nc.vector.tensor_sub(r_t[:, :, 3 * h:3 * h + 3],
                                             hand_t[:, :, 3 * h:3 * h + 3], obj_t[:])

                # ---- d2 = sum_c r^2 (Square on ACT, two strided adds) ----
                sq_t = work.tile([P, E, 6], bf16)
                nc.scalar.activation(sq_t[:], r_t[:], Act.Square)
                sqa = sq_t[:]

                def csum(dst, src_ap, n):
                    """dst[t,h] = src[t,3h]+src[t,3h+1]+src[t,3h+2] over n positions."""
                    v = [bass.AP(tensor=src_ap.tensor, offset=src_ap.offset + cc,
                                 ap=[src_ap.ap[0], [3, 2 * n]]) for cc in range(3)]
                    tmp = work.tile([P, n, 2], bf16, tag=f"csum_tmp")
                    ta = bass.AP(tensor=tmp.tensor, offset=tmp[:].offset,
                                 ap=[tmp[:].ap[0], [1, 2 * n]])
                    nc.vector.tensor_add(ta, v[0], v[1])
                    nc.vector.tensor_add(dst, ta, v[2])

                d2_t = work.tile([P, E, 2], bf16)
                csum(d2_t[:].opt(), sqa, E)
                d_t = work.tile([P, E, 2], bf16)
                nc.scalar.activation(d_t[:], d2_t[:], Act.Sqrt)

                # ---- contact distance partial ----
                derr_t = work.tile([P, W, 2], bf16)
                nc.scalar.activation(derr_t[:], d_t[:, 3:3 + W, :], Act.Abs, bias=c_neg01[:])
                l1p_t = work.tile([P, W, 2], bf16)
                nc.vector.scalar_tensor_tensor(
                    out=l1p_t[:], in0=probs_t[:, 3:3 + W, 0:2], scalar=1.0, in1=derr_t[:],
                    op0=Alu.mult, op1=Alu.mult, accum_out=l1s[:, c:c + 1])

                # ---- velocity ----
                dr_t = work.tile([P, W, 6], bf16)
                nc.vector.tensor_sub(dr_t[:], r_t[:, 4:4 + W, :], r_t[:, 3:3 + W, :])
                dsq_t = work.tile([P, W, 6], bf16)
                nc.scalar.activation(dsq_t[:], dr_t[:], Act.Square)
                v2_t = work.tile([P, W, 2], bf16)
                csum(v2_t[:].opt(), dsq_t[:], W)
                vd_t = work.tile([P, W, 2], bf16)
                nc.scalar.activation(vd_t[:], v2_t[:], Act.Sqrt)
                if c == C - 1:
                    nc.vector.memset(vd_t[H:P, W - 1:W, :], 0.0)  # j=seq-1 invalid
                l2p_t = work.tile([P, W, 2], bf16)
                nc.vector.scalar_tensor_tensor(
                    out=l2p_t[:], in0=probs_t[:, 4:4 + W, 0:2], scalar=1.0, in1=vd_t[:],
                    op0=Alu.mult, op1=Alu.mult, accum_out=l2s[:, c:c + 1])

                # ---- smoothness ----
                e_t = work.tile([P, E - 1, 2], bf16)
                nc.vector.tensor_sub(e_t[:], d_t[:, 1:E, :], d_t[:, 0:E - 1, :])
                sdp_t = work.tile([P, W + 4, 2], bf16)
                nc.vector.tensor_sub(sdp_t[:], e_t[:, 0:W + 4, :], e_t[:, 1:W + 5, :])
                sd_t = work.tile([P, W + 4, 2], bf16)
                nc.scalar.activation(sd_t[:], sdp_t[:], Act.Abs)
                s2_t = work.tile([P, W + 3, 2], bf16)
                nc.vector.tensor_add(s2_t[:], sd_t[:, 0:W + 3, :], sd_t[:, 1:W + 4, :])
                s4_t = work.tile([P, W + 1, 2], bf16)
                nc.vector.tensor_add(s4_t[:], s2_t[:, 0:W + 1, :], s2_t[:, 2:W + 3, :])
                sm5_t = work.tile([P, W, 2], bf16)
                nc.vector.tensor_add(sm5_t[:], s4_t[:, 0:W, :], sd_t[:, 4:W + 4, :])

                # ---- first contact mask + count (DVE; gpsimd is DMA-only) ----
                cb_t = work.tile([P, W + 1, 2], bf16)
                nc.vector.tensor_scalar(
                    out=cb_t[:], in0=probs_t[:, 2:3 + W, 0:2],
                    scalar1=0.5, scalar2=None, op0=Alu.is_gt)
                q_t = work.tile([P, W, 2], bf16)
                nc.vector.tensor_sub(q_t[:], cb_t[:, 1:W + 1, :], cb_t[:, 0:W, :])
                if c == 0:
                    nc.vector.memset(q_t[0:H, 0:3, :], 0.0)  # t<3 (incl. forced-false t=0)
                if c == C - 1:
                    nc.vector.memset(q_t[H:P, W - 3:W, :], 0.0)  # t >= seq-3
                fc_t = work.tile([P, W, 2], bf16)
                nc.vector.tensor_scalar(
                    out=fc_t[:], in0=q_t[:], scalar1=0.0, scalar2=0.0,
                    op0=Alu.max, op1=Alu.add, accum_out=cns[:, c:c + 1])

                smp_t = work.tile([P, W, 2], bf16)
                nc.vector.scalar_tensor_tensor(
                    out=smp_t[:], in0=sm5_t[:], scalar=1.0, in1=fc_t[:],
                    op0=Alu.mult, op1=Alu.mult, accum_out=sms[:, c:c + 1])

            # ---- final per-partition combine + store ----
            for i, slot in enumerate((l1s, l2s, sms, cns)):
                nc.vector.tensor_reduce(outt[:, i:i + 1], slot[:], axis=mybir.AxisListType.X, op=Alu.add)
            nc.sync.dma_start(out=partials.ap(), in_=outt[:])

    nc.compile()
    return nc


_cache = {}


def _get_nc(bs_local, seq, W):
    key = (bs_local, seq, W)
    if key not in _cache:
        _cache[key] = build_nc(bs_local, seq, W)
    return _cache[key]


def combine_partials(parts, bs, seq, training_step):
    """parts: float array [..., 4] of per-core/per-partition partial sums."""
    s = np.asarray(parts, dtype=np.float64).reshape(-1, 4).sum(axis=0)
    l1 = s[0] / (bs * seq * 2)
    l2 = s[1] / (bs * (seq - 1) * 2) if seq > 1 else 0.0
    cnt = s[3]
    sm = (s[2] / 5.0) / max(cnt, 1.0) if (seq > 5 and cnt > 0) else 0.0
    ramp = min(1.0, float(training_step) / 1000.0)
    return np.array(ramp * (1.0 * l1 + 0.5 * l2 + 0.3 * sm), dtype=np.float32)


def _run(pred_hand_pos, pred_obj_pos, contact_probs, **spmd_kwargs):
    from concourse.bass_utils import run_bass_kernel_spmd

    hand = np.ascontiguousarray(np.asarray(pred_hand_pos, dtype=np.float32))
    obj = np.ascontiguousarray(np.asarray(pred_obj_pos, dtype=np.float32))
    probs = np.ascontiguousarray(np.asarray(contact_probs, dtype=np.float32))
    bs, seq = hand.shape[:2]
    bs_local = bs // N_CORES
    nc = _get_nc(bs_local, seq, W_FULL)

    in_maps = []
    for i in range(N_CORES):
        sl = slice(i * bs_local, (i + 1) * bs_local)
        in_maps.append({
            "pred_hand_pos": hand[sl],
            "pred_obj_pos": obj[sl],
            "contact_probs": probs[sl],
        })
    # The axon terminal occasionally reports the exec unit unrecoverable on
    # the first touch after a previous process's teardown; a retry lands on a
    # recovered device.
    last_err = None
    for _ in range(3):
        try:
            res = run_bass_kernel_spmd(
                nc, in_maps, core_ids=list(range(N_CORES)), **spmd_kwargs
            )
            parts = np.stack([res.results[i]["partials"] for i in range(N_CORES)])
            return parts, res
        except Exception as e:  # noqa: BLE001
            last_err = e
    raise last_err


def kernel(pred_hand_pos, pred_obj_pos, contact_probs, training_step):
    bs, seq = np.asarray(pred_hand_pos).shape[:2]
    parts, _ = _run(pred_hand_pos, pred_obj_pos, contact_probs)
    return combine_partials(parts, bs, seq, training_step)


# revision 10
# speedup vs baseline: 2.5087x; 1.2525x over previous
"""ContactAwareLoss Trainium2 kernel.

Strategy: pure data-parallel over batch (512 rows -> 8 cores x 64 rows).
Each core computes four partial sums over its shard:
  [0] sum_{t,h} probs2 * |dist - 0.1|            (contact distance, unnormalized)
  [1] sum_{j,h} probs2[j+1] * ||r[j+1]-r[j]||     (contact velocity, unnormalized)
  [2] sum_{t,h} first_contact * (5-tap sum of |second diff of dist|)
  [3] sum first_contact                           (count)
The host divides by the global element counts / count and applies the ramp.

On-chip layout: partition p = half*64 + b  (sequence halved so 64 batch rows
fill 128 partitions); free dim = time within the half, processed in W-wide
chunks with a 3-element halo on both sides.  The halo at the half boundary is
filled with real neighbour data via small extra DMAs; the halo at the global
sequence ends is zero-filled and the affected contributions are masked by
zeroing q/vd edge columns (smoothness valid t in [3, seq-3), velocity valid
j in [0, seq-1)).

Engine split (v2):
 - DMA: the three big per-chunk input loads ride SWDGE (nc.gpsimd.dma_start).
   SWDGE sprays descriptors across all 16 SDMA engines (~120+ GB/s measured)
   while the HWDGE rings (sync/scalar) pin all >4KB descriptors to SDMA
   engines 64/65 only (~52 GB/s aggregate).  Halo loads (tiny) stay on the
   HWDGE rings so they don't consume Q7 descriptor-emission throughput.
 - GpSimd: SWDGE descriptor emission only (no compute -- gpsimd tensor ops
   measured ~8 G elem/s, 15x slower than DVE).
 - DVE: diffs/movsum in bf16 2x mode, abs via tensor_scalar abs_max (4x),
   first-contact mask (is_gt), fused weighted-sum accumulators.
 - ScalarE: Square / Sqrt only (contiguous APs).
"""

import numpy as np

BS, SEQ = 512, 4096
N_CORES = 8
W_FULL = 512  # chunk width (per half-sequence)
R_BROADCAST = True  # single hand-obj sub via zero-stride obj AP


def build_nc(bs_local, seq, W):
    import concourse.bass as bass
    import concourse.bacc as bacc
    import concourse.tile as tile
    from concourse import mybir

    f32 = mybir.dt.float32
    bf16 = mybir.dt.bfloat16
    Alu = mybir.AluOpType
    Act = mybir.ActivationFunctionType

    P = 2 * bs_local          # partitions used
    HS = seq // 2             # timesteps per partition row
    assert HS % W == 0
    C = HS // W               # chunks
    E = W + 6                 # chunk width incl. +-3 halo
    H = P // 2

    nc = bacc.Bacc("TRN2", target_bir_lowering=False, debug=False)
    hand = nc.dram_tensor("pred_hand_pos", [bs_local, seq, 2, 3], f32, kind="ExternalInput")
    obj = nc.dram_tensor("pred_obj_pos", [bs_local, seq, 3], f32, kind="ExternalInput")
    probs = nc.dram_tensor("contact_probs", [bs_local, seq, 3], f32, kind="ExternalInput")
    partials = nc.dram_tensor("partials", [P, 4], f32, kind="ExternalOutput")

    def dram_ap(t, offset, dims):
        return bass.AP(tensor=t, offset=offset, ap=[list(d) for d in dims])

    with tile.TileContext(nc) as tc:
        import contextlib
        with contextlib.ExitStack() as ctx:
            inp = ctx.enter_context(tc.tile_pool(name="inp", bufs=2))
            work = ctx.enter_context(tc.tile_pool(name="work", bufs=1))
            singles = ctx.enter_context(tc.tile_pool(name="singles", bufs=1))
            psum = ctx.enter_context(tc.psum_pool(name="ps", bufs=1))

            l1s = singles.tile([P, C], f32)
            l2s = singles.tile([P, C], f32)
            sms = singles.tile([P, C], f32)
            cns = singles.tile([P, C], f32)
            outt = singles.tile([P, 4], f32)
            c_neg01 = singles.tile([P, 1], f32)
            nc.vector.memset(c_neg01[:], -0.1)

            # identity weights for TensorE c-sum matmuls: ident[p, j] = (j - p == 0)
            iota_t = singles.tile([P, P], mybir.dt.int32)
            nc.gpsimd.iota(iota_t[:], pattern=[[1, P]], base=0, channel_multiplier=-1)
            ident = singles.tile([P, P], bf16)
            nc.vector.tensor_scalar(out=ident[:], in0=iota_t[:], scalar1=0,
                                    scalar2=None, op0=Alu.is_equal)

            def csum_mm(psum_out, src_ap, n):
                """psum_out[p, j] = src[p, 3j] + src[p, 3j+1] + src[p, 3j+2], j < 2n.

                TensorE identity matmuls, accumulated in PSUM; groups of <=512
                output columns keep each matmul inside one PSUM bank.
                """
                ncols = 2 * n
                g0 = 0
                while g0 < ncols:
                    g1 = min(g0 + 512, ncols)
                    for cc in range(3):
                        rhs = bass.AP(tensor=src_ap.tensor,
                                      offset=src_ap.offset + 3 * g0 + cc,
                                      ap=[src_ap.ap[0], [3, g1 - g0]])
                        nc.tensor.matmul(out=psum_out[:, g0:g1],
                                         lhsT=ident[:], rhs=rhs,
                                         start=(cc == 0), stop=(cc == 2))
                    g0 = g1

            for c in range(C):
                t0 = c * W  # first owned timestep (within half)
                t_lo = max(0, t0 - 3)
                t_hi = min(HS, t0 + W + 3)
                col_lo = t_lo - (t0 - 3)
                ncols = t_hi - t_lo

                hand_t = inp.tile([P, E, 6], f32)
                obj_t = inp.tile([P, E, 3], f32)
                probs_t = inp.tile([P, E, 3], f32)

                loads = (
                    (hand_t, hand, 6, nc.sync),
                    (obj_t, obj, 3, nc.sync),
                    (probs_t, probs, 3, nc.scalar),
                )
                for tile_buf, ten, k, halo_eng in loads:
                    # big main loads ride SWDGE (sprays SDMA engine pairs,
                    # round-robin per instruction).  The hand tensor is twice
                    # the size of obj/probs, so it is split into two half-
                    # batch loads keeping all SWDGE instructions equal-sized
                    # -- unequal loads leave the engine pairs imbalanced.
                    if k == 6:
                        for half in range(2):
                            nc.gpsimd.dma_start(
                                out=tile_buf[half * H:(half + 1) * H,
                                             col_lo:col_lo + ncols, :],
                                in_=dram_ap(ten, (half * HS + t_lo) * k,
                                            [[seq * k, bs_local], [1, ncols * k]]),
                            )
                    else:
                        nc.gpsimd.dma_start(
                            out=tile_buf[:, col_lo:col_lo + ncols, :],
                            in_=dram_ap(ten, t_lo * k,
                                        [[HS * k, 2], [seq * k, bs_local], [1, ncols * k]]),
                        )
                    if c == 0:
                        halo_eng.dma_start(
                            out=tile_buf[H:P, 0:3, :],
                            in_=dram_ap(ten, (HS - 3) * k,
                                        [[seq * k, bs_local], [1, 3 * k]]),
                        )
                        nc.vector.memset(tile_buf[0:H, 0:3, :], 0.0)
                    if c == C - 1:
                        halo_eng.dma_start(
                            out=tile_buf[0:H, W + 3:E, :],
                            in_=dram_ap(ten, HS * k,
                                        [[seq * k, bs_local], [1, 3 * k]]),
                        )
                        nc.vector.memset(tile_buf[H:P, W + 3:E, :], 0.0)

                # ---- r = hand - obj (fp32 -> bf16) ----
                r_t = work.tile([P, E, 6], bf16)
                if R_BROADCAST:
                    ha = hand_t[:]
                    ra = r_t[:]
                    oa = obj_t[:]
                    hand_v = bass.AP(tensor=ha.tensor, offset=ha.offset,
                                     ap=[ha.ap[0], [6, E], [3, 2], [1, 3]])
                    r_v = bass.AP(tensor=ra.tensor, offset=ra.offset,
                                  ap=[ra.ap[0], [6, E], [3, 2], [1, 3]])
                    obj_v = bass.AP(tensor=oa.tensor, offset=oa.offset,
                                    ap=[oa.ap[0], [3, E], [0, 2], [1, 3]])
                    nc.vector.tensor_sub(r_v, hand_v, obj_v)
                else:
                    for h in range(2):
                        nc.vector.tensor_sub(r_t[:, :, 3 * h:3 * h + 3],
                                             hand_t[:, :, 3 * h:3 * h + 3], obj_t[:])

                # ---- d2 = sum_c r^2 (Square on ACT, c-sum on TensorE) ----
                sq_t = work.tile([P, E, 6], bf16)
                nc.scalar.activation(sq_t[:], r_t[:], Act.Square)

                d2_ps = psum.tile([P, 2 * E], f32)
                csum_mm(d2_ps, sq_t[:].opt(), E)
                d_t = work.tile([P, E, 2], bf16)
                nc.scalar.activation(d_t[:].opt(), d2_ps[:], Act.Sqrt)

                # ---- contact distance partial ----
                derr_t = work.tile([P, W, 2], bf16)
                nc.scalar.activation(derr_t[:], d_t[:, 3:3 + W, :], Act.Abs, bias=c_neg01[:])
                l1p_t = work.tile([P, W, 2], bf16)
                nc.vector.scalar_tensor_tensor(
                    out=l1p_t[:], in0=probs_t[:, 3:3 + W, 0:2], scalar=1.0, in1=derr_t[:],
                    op0=Alu.mult, op1=Alu.mult, accum_out=l1s[:, c:c + 1])

                # ---- velocity ----
                dr_t = work.tile([P, W, 6], bf16)
                nc.vector.tensor_sub(dr_t[:], r_t[:, 4:4 + W, :], r_t[:, 3:3 + W, :])
                dsq_t = work.tile([P, W, 6], bf16)
                nc.scalar.activation(dsq_t[:], dr_t[:], Act.Square)
                v2_ps = psum.tile([P, 2 * W], f32)
                csum_mm(v2_ps, dsq_t[:].opt(), W)
                vd_t = work.tile([P, W, 2], bf16)
                nc.scalar.activation(vd_t[:].opt(), v2_ps[:], Act.Sqrt)
                if c == C - 1:
                    nc.vector.memset(vd_t[H:P, W - 1:W, :], 0.0)  # j=seq-1 invalid
                l2p_t = work.tile([P, W, 2], bf16)
                nc.vector.scalar_tensor_tensor(
                    out=l2p_t[:], in0=probs_t[:, 4:4 + W, 0:2], scalar=1.0, in1=vd_t[:],
                    op0=Alu.mult, op1=Alu.mult, accum_out=l2s[:, c:c + 1])

                # ---- smoothness ----
                e_t = work.tile([P, E - 1, 2], bf16)
                nc.vector.tensor_sub(e_t[:], d_t[:, 1:E, :], d_t[:, 0:E - 1, :])
                sdp_t = work.tile([P, W + 4, 2], bf16)
                nc.vector.tensor_sub(sdp_t[:], e_t[:, 0:W + 4, :], e_t[:, 1:W + 5, :])
                sd_t = work.tile([P, W + 4, 2], bf16)
                nc.scalar.activation(sd_t[:], sdp_t[:], Act.Abs)
                s2_t = work.tile([P, W + 3, 2], bf16)
                nc.vector.tensor_add(s2_t[:], sd_t[:, 0:W + 3, :], sd_t[:, 1:W + 4, :])
                s4_t = work.tile([P, W + 1, 2], bf16)
                nc.vector.tensor_add(s4_t[:], s2_t[:, 0:W + 1, :], s2_t[:, 2:W + 3, :])
                sm5_t = work.tile([P, W, 2], bf16)
                nc.vector.tensor_add(sm5_t[:], s4_t[:, 0:W, :], sd_t[:, 4:W + 4, :])

                # ---- first contact mask + count (DVE; gpsimd is DMA-only) ----
                cb_t = work.tile([P, W + 1, 2], bf16)
                nc.vector.tensor_scalar(
                    out=cb_t[:], in0=probs_t[:, 2:3 + W, 0:2],
                    scalar1=0.5, scalar2=None, op0=Alu.is_gt)
                q_t = work.tile([P, W, 2], bf16)
                nc.vector.tensor_sub(q_t[:], cb_t[:, 1:W + 1, :], cb_t[:, 0:W, :])
                if c == 0:
                    nc.vector.memset(q_t[0:H, 0:3, :], 0.0)  # t<3 (incl. forced-false t=0)
                if c == C - 1:
                    nc.vector.memset(q_t[H:P, W - 3:W, :], 0.0)  # t >= seq-3
                fc_t = work.tile([P, W, 2], bf16)
                nc.vector.tensor_scalar(
                    out=fc_t[:], in0=q_t[:], scalar1=0.0, scalar2=0.0,
                    op0=Alu.max, op1=Alu.add, accum_out=cns[:, c:c + 1])

                smp_t = work.tile([P, W, 2], bf16)
                nc.vector.scalar_tensor_tensor(
                    out=smp_t[:], in0=sm5_t[:], scalar=1.0, in1=fc_t[:],
                    op0=Alu.mult, op1=Alu.mult, accum_out=sms[:, c:c + 1])

            # ---- final per-partition combine + store ----
            for i, slot in enumerate((l1s, l2s, sms, cns)):
                nc.vector.tensor_reduce(outt[:, i:i + 1], slot[:], axis=mybir.AxisListType.X, op=Alu.add)
            nc.sync.dma_start(out=partials.ap(), in_=outt[:])

    nc.compile()
    return nc


_cache = {}


def _get_nc(bs_local, seq, W):
    key = (bs_local, seq, W)
    if key not in _cache:
        _cache[key] = build_nc(bs_local, seq, W)
    return _cache[key]


def combine_partials(parts, bs, seq, training_step):
    """parts: float array [..., 4] of per-core/per-partition partial sums."""
    s = np.asarray(parts, dtype=np.float64).reshape(-1, 4).sum(axis=0)
    l1 = s[0] / (bs * seq * 2)
    l2 = s[1] / (bs * (seq - 1) * 2) if seq > 1 else 0.0
    cnt = s[3]
    sm = (s[2] / 5.0) / max(cnt, 1.0) if (seq > 5 and cnt > 0) else 0.0
    ramp = min(1.0, float(training_step) / 1000.0)
    return np.array(ramp * (1.0 * l1 + 0.5 * l2 + 0.3 * sm), dtype=np.float32)


def _run(pred_hand_pos, pred_obj_pos, contact_probs, **spmd_kwargs):
    from concourse.bass_utils import run_bass_kernel_spmd

    hand = np.ascontiguousarray(np.asarray(pred_hand_pos, dtype=np.float32))
    obj = np.ascontiguousarray(np.asarray(pred_obj_pos, dtype=np.float32))
    probs = np.ascontiguousarray(np.asarray(contact_probs, dtype=np.float32))
    bs, seq = hand.shape[:2]
    bs_local = bs // N_CORES
    nc = _get_nc(bs_local, seq, W_FULL)

    in_maps = []
    for i in range(N_CORES):
        sl = slice(i * bs_local, (i + 1) * bs_local)
        in_maps.append({
            "pred_hand_pos": hand[sl],
            "pred_obj_pos": obj[sl],
            "contact_probs": probs[sl],
        })
    # The axon terminal occasionally reports the exec unit unrecoverable on
    # the first touch after a previous process's teardown; a retry lands on a
    # recovered device.
    last_err = None
    for _ in range(3):
        try:
            res = run_bass_kernel_spmd(
                nc, in_maps, core_ids=list(range(N_CORES)), **spmd_kwargs
            )
            parts = np.stack([res.results[i]["partials"] for i in range(N_CORES)])
            return parts, res
        except Exception as e:  # noqa: BLE001
            last_err = e
    raise last_err


def kernel(pred_hand_pos, pred_obj_pos, contact_probs, training_step):
    bs, seq = np.asarray(pred_hand_pos).shape[:2]
    parts, _ = _run(pred_hand_pos, pred_obj_pos, contact_probs)
    return combine_partials(parts, bs, seq, training_step)
